# revision 11
# baseline (speedup 1.0000x reference)
# Block-circulant linear kernel for Trainium2 (Bass/Tile), 8-core SPMD.
#
# y[b, 16m+p] = sum_{n,q} blocks[(m-n)%512, p, q] * x[b, 16n+q]
#
# CRT split (exact): z^512-1 = (z^256-1)(z^256+1). With
#   Xc[n] = x[n] + x[n+256],  Xs[n] = x[n] - x[n+256]   (block index n)
#   Bc[s] = B[s] + B[s+256],  Bs[s] = B[s] - B[s+256]
# the problem becomes a 256-cyclic conv (Bc*Xc) and a 256-negacyclic conv
# (Bs*Xs); y[m] = (Yc[m]+Ys[m])/2, y[m+256] = (Yc[m]-Ys[m])/2. The fold /
# unfold are O(input) host-side adds; the device still performs the full
# contraction, at half the MACs and ~0.57x the DMA bytes of the direct form.
#
# Sharding: cores 0-3 compute Yc (64 output block-rows each), cores 4-7
# compute Ys. Negacyclic wrap signs are folded into the host-built BIGQ
# window, so all 8 cores run the SAME program on different data.
#
# Per core, a doubled+shifted "BIGQ" layout of the 256 folded blocks lives
# in SBUF:  BIGQ[(ni,q), u*16+p] = sgn * Bh[(m0 + u - ni) % 256, p, q]
# so every 128x128 weight tile of the implied circulant matrix is a
# contiguous 128-column slice. All (m_tile t, n_chunk c) pairs with the
# same diagonal d = t - c share one stationary tile: 39 accumulating
# matmuls into a single PSUM bank [128 mp, 8 t x 32 b].
#
# The xt layout is reversed (c' = 31 - c) and the psum t axis flipped
# (t' = 7 - t) so both streams are consumed in DMA arrival order.
import numpy as np

B = 32
NB = 512          # number of 16x16 blocks in the original problem
L = 256           # folded subproblem length (cyclic / negacyclic)
NCORES = 8
MBLK = L // 4     # 64 output block-rows per core (4 cores per subproblem)
NCH_C = L // 8    # 32 n-chunks of 128 contraction rows
ND = NCH_C + 7    # 39 diagonal offsets d in [-31, 7]
W = 8 * ND + 8    # 320 BIGQ window width (in u units of 16 columns)
XT_COLS = NCH_C * B   # 1024

DTYPE = "float16"
NWARM = 4   # logical fp32 warm-up matmuls (clock-boost trigger)

_cached = {}
_last_results = None  # BassKernelResults of the most recent run (for profiling)


def _np_dtype(name):
    if name == "bfloat16":
        import ml_dtypes

        return ml_dtypes.bfloat16
    if name == "float16":
        return np.float16
    return np.float32


def _split_dt(dt_name):
    if ":" in dt_name:
        wt, mv = dt_name.split(":")
        return wt, mv
    return dt_name, dt_name


def _build_program(dt_name):
    import concourse.bacc as bacc
    import concourse.mybir as mybir
    import concourse.tile as tile

    wt_name, mv_name = _split_dt(dt_name)
    wdt = getattr(mybir.dt, wt_name)
    mdt = getattr(mybir.dt, mv_name)
    f32 = mybir.dt.float32

    nc = bacc.Bacc("TRN2", target_bir_lowering=False, debug=False, num_devices=NCORES)
    xt_d = nc.declare_dram_parameter("xt", [128, XT_COLS], mdt, isOutput=False)
    bq_d = nc.declare_dram_parameter("bigq", [128, W * 16], wdt, isOutput=False)
    out_d = nc.declare_dram_parameter("out", [128, 256], mdt, isOutput=True)

    # graduated bigq chunks: small first (early PE start), large later
    # (fewer DIRECT2D generations, which cost ~0.65us each on a DGE).
    bq_cuts = [0, 512, 1280, 2560, 3840, 5120]

    with tile.TileContext(nc) as tc:
        with (
            tc.tile_pool(name="data", bufs=1) as data_pool,
            tc.tile_pool(name="psum", bufs=1, space="PSUM") as psum_pool,
        ):
            xt = data_pool.tile([128, XT_COLS], mdt)
            bq = data_pool.tile([128, W * 16], wdt)
            out_sb = data_pool.tile([128, 256], mdt)
            warm_sb = data_pool.tile([128, 256], mdt)
            acc = psum_pool.tile([128, 256], f32)
            warm_ps = psum_pool.tile([128, 256], f32)

            # two HWDGE issue engines (sync=SP, scalar=ACT), streams queued
            # in consumption order: bq0+xt0 first.
            half = XT_COLS // 2
            nc.scalar.dma_start(bq[:, bq_cuts[0]:bq_cuts[1]],
                                bq_d[:, bq_cuts[0]:bq_cuts[1]])
            nc.sync.dma_start(xt[:, 0:half], xt_d[:, 0:half])
            nc.sync.dma_start(bq[:, bq_cuts[1]:bq_cuts[2]],
                              bq_d[:, bq_cuts[1]:bq_cuts[2]])
            nc.scalar.dma_start(xt[:, half:XT_COLS], xt_d[:, half:XT_COLS])
            nc.scalar.dma_start(bq[:, bq_cuts[2]:bq_cuts[3]],
                                bq_d[:, bq_cuts[2]:bq_cuts[3]])
            nc.sync.dma_start(bq[:, bq_cuts[3]:bq_cuts[4]],
                              bq_d[:, bq_cuts[3]:bq_cuts[4]])
            nc.scalar.dma_start(bq[:, bq_cuts[4]:bq_cuts[5]],
                                bq_d[:, bq_cuts[4]:bq_cuts[5]])

            # PE warm-up while DMA streams in: starts the HAM activity
            # window (2.4GHz after ~3.4us of PE busy) without delaying the
            # real stream past first-chunk arrival.
            nwarm = NWARM if mv_name in ("float16", "bfloat16") else 0
            if nwarm:
                nc.gpsimd.memset(warm_sb[:], 0.0)
            for wi in range(nwarm):
                nc.tensor.matmul(
                    warm_ps[:], warm_sb[:, 0:128], warm_sb[:],
                    start=(wi == 0), stop=(wi == nwarm - 1),
                )

            # d = t - c diagonal; stationary tile = BIGQ cols [16u0, 16u0+128)
            for i in range(ND):
                d = i - (NCH_C - 1)
                u0 = 8 * i + 8
                t_lo = max(0, d)
                t_hi = min(7, NCH_C - 1 + d)
                nt = t_hi - t_lo + 1
                tp_lo = 7 - t_hi                 # flipped psum tile index
                cp_lo = NCH_C - 1 + d - t_hi     # reversed xt chunk index
                nc.tensor.matmul(
                    acc[:, 32 * tp_lo: 32 * (tp_lo + nt)],
                    bq[:, 16 * u0: 16 * u0 + 128],
                    xt[:, 32 * cp_lo: 32 * (cp_lo + nt)],
                    start=(i == 0),   # clears the whole PSUM bank
                    stop=(i == ND - 1),
                )
                if i == ND - 5:
                    # psum tiles t=0..3 (cols 128:256) got their last
                    # accumulation at i = NCH_C-1+t <= ND-5; cast them out
                    # while the remaining diagonals accumulate cols 0:128.
                    nc.scalar.copy(out_sb[:, 128:256], acc[:, 128:256])
                    nc.scalar.dma_start(out_d[:, 128:256], out_sb[:, 128:256])

            # remaining half: cast + partition-split DMA (64 descriptors
            # per DGE halves the descriptor-generation latency).
            nc.vector.tensor_copy(out_sb[:, 0:128], acc[:, 0:128])
            nc.sync.dma_start(out_d[0:64, 0:128], out_sb[0:64, 0:128])
            nc.scalar.dma_start(out_d[64:128, 0:128], out_sb[64:128, 0:128])
    nc.compile()
    return nc


def _get_program(dt_name):
    key = (dt_name, NWARM)
    if key not in _cached:
        _cached[key] = _build_program(dt_name)
    return _cached[key]


def _xt_layout(xh):
    """[32, 16L] half -> [128, XT_COLS]: xt[(ni*16+q), c'*32+b] with
    c' = NCH_C-1-c reversed chunk order."""
    xt = (
        xh.T.reshape(NCH_C, 128, B).transpose(1, 0, 2)[:, ::-1, :]
        .reshape(128, XT_COLS)
    )
    return np.ascontiguousarray(xt)


def _prep_inputs(x, blocks, dt_name):
    """Host-side fold + layout prep (numpy ops on the small inputs)."""
    x = np.ascontiguousarray(np.asarray(x), dtype=np.float32)
    blocks = np.ascontiguousarray(np.asarray(blocks), dtype=np.float32)
    wt_name, mv_name = _split_dt(dt_name)
    np_w, np_m = _np_dtype(wt_name), _np_dtype(mv_name)

    xc = x[:, : 16 * L] + x[:, 16 * L:]
    xs = x[:, : 16 * L] - x[:, 16 * L:]
    bc = blocks[:L] + blocks[L:]
    bs = blocks[:L] - blocks[L:]

    xt_c = _xt_layout(xc).astype(np_m)
    xt_s = _xt_layout(xs).astype(np_m)

    u = np.arange(W)
    ni = np.arange(8)
    j = u[None, :] - ni[:, None]                    # [8, W], in [-7, W-1]
    in_maps = []
    for k in range(NCORES):
        neg = k >= 4
        m0 = (k % 4) * MBLK
        jj = m0 + j                                  # in [-7, 511]
        idx = jj % L
        bh = bs if neg else bc
        bigq = bh[idx]                               # [8, W, p, q]
        if neg:
            # window index jj = (m-n) + L, so wrap sign flips at jj == L
            sgn = np.where(jj >= L, 1.0, -1.0).astype(np.float32)
            bigq = bigq * sgn[:, :, None, None]
        bigq = bigq.transpose(0, 3, 1, 2).reshape(128, W * 16)  # [(ni,q),(u,p)]
        in_maps.append({
            "xt": xt_s if neg else xt_c,
            "bigq": np.ascontiguousarray(bigq.astype(np_w)),
        })
    return in_maps


def _assemble(results):
    """Per-core [128 (mi,p), 256 (t',b)] -> slabs -> CRT unfold."""
    yc = np.empty((B, 16 * L), dtype=np.float32)
    ys = np.empty((B, 16 * L), dtype=np.float32)
    for k in range(NCORES):
        o = np.asarray(results[k]["out"]).astype(np.float32)
        slab = (
            o.reshape(128, 8, B)[:, ::-1, :].transpose(2, 1, 0).reshape(B, 1024)
        )
        dst = ys if k >= 4 else yc
        kk = k % 4
        dst[:, 1024 * kk: 1024 * (kk + 1)] = slab
    y = np.empty((B, NB * 16), dtype=np.float32)
    y[:, : 16 * L] = 0.5 * (yc + ys)
    y[:, 16 * L:] = 0.5 * (yc - ys)
    return y


def kernel(x, blocks):
    global _last_results
    from concourse.bass_utils import run_bass_kernel_spmd

    nc = _get_program(DTYPE)
    in_maps = _prep_inputs(x, blocks, DTYPE)
    res = run_bass_kernel_spmd(nc, in_maps, list(range(NCORES)))
    _last_results = res
    return _assemble(res.results)


# revision 15
# speedup vs baseline: 1.2452x; 1.2452x over previous
# Block-circulant linear kernel for Trainium2 (Bass/Tile), 8-core SPMD.
#
# y[b, 16m+p] = sum_{n,q} blocks[(m-n)%512, p, q] * x[b, 16n+q]
#
# CRT split (exact): z^512-1 = (z^256-1)(z^256+1). With
#   Xc[n] = x[n] + x[n+256],  Xs[n] = x[n] - x[n+256]   (block index n)
#   Bc[s] = B[s] + B[s+256],  Bs[s] = B[s] - B[s+256]
# the problem becomes a 256-cyclic conv (Bc*Xc) and a 256-negacyclic conv
# (Bs*Xs); y[m] = (Yc[m]+Ys[m])/2, y[m+256] = (Yc[m]-Ys[m])/2. The fold /
# unfold are O(input) host-side adds; the device still performs the full
# contraction, at half the MACs and ~0.57x the DMA bytes of the direct form.
#
# Sharding: cores 0-3 compute Yc (64 output block-rows each), cores 4-7
# compute Ys. Negacyclic wrap signs are folded into the host-built BIGQ
# window, so all 8 cores run the SAME program on different data.
#
# Per core, a doubled+shifted "BIGQ" layout of the 256 folded blocks lives
# in SBUF:  BIGQ[(ni,q), u*16+p] = sgn * Bh[(m0 + u - ni) % 256, p, q]
# so every 128x128 weight tile of the implied circulant matrix is a
# contiguous 128-column slice. All (m_tile t, n_chunk c) pairs with the
# same diagonal d = t - c share one stationary tile: 39 accumulating
# matmuls into a single PSUM bank [128 mp, 8 t x 32 b].
#
# The xt layout is reversed (c' = 31 - c) and the psum t axis flipped
# (t' = 7 - t) so both streams are consumed in DMA arrival order.
import numpy as np

B = 32
NB = 512          # number of 16x16 blocks in the original problem
L = 256           # folded subproblem length (cyclic / negacyclic)
NCORES = 8
MBLK = L // 4     # 64 output block-rows per core (4 cores per subproblem)
NCH_C = L // 8    # 32 n-chunks of 128 contraction rows
ND = NCH_C + 7    # 39 diagonal offsets d in [-31, 7]
W = 8 * ND + 8    # 320 BIGQ window width (in u units of 16 columns)
XT_COLS = NCH_C * B   # 1024

DTYPE = "float16"
NWARM = 3   # warm-up pairs: 2*NWARM fp32 matmul instrs (clock-boost trigger)

_cached = {}
_last_results = None  # BassKernelResults of the most recent run (for profiling)


def _np_dtype(name):
    if name == "bfloat16":
        import ml_dtypes

        return ml_dtypes.bfloat16
    if name == "float16":
        return np.float16
    return np.float32


def _split_dt(dt_name):
    if ":" in dt_name:
        wt, mv = dt_name.split(":")
        return wt, mv
    return dt_name, dt_name


def _build_program(dt_name):
    import concourse.bacc as bacc
    import concourse.mybir as mybir
    import concourse.tile as tile

    wt_name, mv_name = _split_dt(dt_name)
    wdt = getattr(mybir.dt, wt_name)
    mdt = getattr(mybir.dt, mv_name)
    f32 = mybir.dt.float32

    nc = bacc.Bacc("TRN2", target_bir_lowering=False, debug=False, num_devices=NCORES)
    xt_d = nc.declare_dram_parameter("xt", [128, XT_COLS], mdt, isOutput=False)
    bq_d = nc.declare_dram_parameter("bigq", [128, W * 16], wdt, isOutput=False)
    out_d = nc.declare_dram_parameter("out", [128, 256], mdt, isOutput=True)

    # bigq chunks: ~1024 cols each. Chunk-completion semaphores fire ~1.2us
    # after the last byte (HBM write-receipt round trip), so the PE stream
    # trails the DMA stream by that latency; 5 chunks keeps the DGE
    # generation pipeline (~0.65us/chunk/ring) off the critical path.
    bq_cuts = [0, 1024, 2048, 3072, 4096, 5120]

    with tile.TileContext(nc) as tc:
        with (
            tc.tile_pool(name="data", bufs=1) as data_pool,
            tc.tile_pool(name="psum", bufs=1, space="PSUM") as psum_pool,
        ):
            xt = data_pool.tile([128, XT_COLS], mdt)
            bq = data_pool.tile([128, W * 16], wdt)
            out_sb = data_pool.tile([128, 256], mdt)
            warm_sb = data_pool.tile([128, 256], f32)
            acc = psum_pool.tile([128, 256], f32)
            warm_ps = psum_pool.tile([128, 256], f32)

            # two HWDGE issue engines (sync=SP, scalar=ACT), streams queued
            # in consumption order: bq0+xt0 first.
            half = XT_COLS // 2
            nc.scalar.dma_start(bq[:, bq_cuts[0]:bq_cuts[1]],
                                bq_d[:, bq_cuts[0]:bq_cuts[1]])
            nc.sync.dma_start(xt[:, 0:half], xt_d[:, 0:half])
            nc.sync.dma_start(bq[:, bq_cuts[1]:bq_cuts[2]],
                              bq_d[:, bq_cuts[1]:bq_cuts[2]])
            nc.scalar.dma_start(xt[:, half:XT_COLS], xt_d[:, half:XT_COLS])
            nc.scalar.dma_start(bq[:, bq_cuts[2]:bq_cuts[3]],
                                bq_d[:, bq_cuts[2]:bq_cuts[3]])
            nc.sync.dma_start(bq[:, bq_cuts[3]:bq_cuts[4]],
                              bq_d[:, bq_cuts[3]:bq_cuts[4]])
            nc.scalar.dma_start(bq[:, bq_cuts[4]:bq_cuts[5]],
                                bq_d[:, bq_cuts[4]:bq_cuts[5]])

            # PE warm-up while DMA streams in: fp32 N=256 matmuls (~430ns
            # each) keep the PE continuously busy so the HAM clock gate
            # flips to 2.4GHz (needs ~3.4us sustained); sized to end right
            # around the first chunk-completion semaphore (~10.3us).
            nwarm = 2 * NWARM if mv_name in ("float16", "bfloat16") else 0
            if nwarm:
                nc.gpsimd.memset(warm_sb[:], 0.0)
            for wi in range(nwarm):
                nc.tensor.matmul(
                    warm_ps[:], warm_sb[:, 0:128], warm_sb[:],
                    start=(wi == 0), stop=(wi == nwarm - 1),
                )

            # d = t - c diagonal; stationary tile = BIGQ cols [16u0, 16u0+128)
            for i in range(ND):
                d = i - (NCH_C - 1)
                u0 = 8 * i + 8
                t_lo = max(0, d)
                t_hi = min(7, NCH_C - 1 + d)
                nt = t_hi - t_lo + 1
                tp_lo = 7 - t_hi                 # flipped psum tile index
                cp_lo = NCH_C - 1 + d - t_hi     # reversed xt chunk index
                nc.tensor.matmul(
                    acc[:, 32 * tp_lo: 32 * (tp_lo + nt)],
                    bq[:, 16 * u0: 16 * u0 + 128],
                    xt[:, 32 * cp_lo: 32 * (cp_lo + nt)],
                    start=(i == 0),   # clears the whole PSUM bank
                    stop=(i == ND - 1),
                )
                if i == ND - 5:
                    # psum tiles t=0..3 (cols 128:256) got their last
                    # accumulation at i = NCH_C-1+t <= ND-5; cast them out
                    # while the remaining diagonals accumulate cols 0:128.
                    nc.scalar.copy(out_sb[:, 128:256], acc[:, 128:256])
                    nc.scalar.dma_start(out_d[:, 128:256], out_sb[:, 128:256])

            # remaining half: cast + partition-split DMA (64 descriptors
            # per DGE halves the descriptor-generation latency).
            nc.vector.tensor_copy(out_sb[:, 0:128], acc[:, 0:128])
            nc.sync.dma_start(out_d[0:64, 0:128], out_sb[0:64, 0:128])
            nc.scalar.dma_start(out_d[64:128, 0:128], out_sb[64:128, 0:128])
    nc.compile()
    return nc


def _get_program(dt_name):
    key = (dt_name, NWARM)
    if key not in _cached:
        _cached[key] = _build_program(dt_name)
    return _cached[key]


def _xt_layout(xh):
    """[32, 16L] half -> [128, XT_COLS]: xt[(ni*16+q), c'*32+b] with
    c' = NCH_C-1-c reversed chunk order."""
    xt = (
        xh.T.reshape(NCH_C, 128, B).transpose(1, 0, 2)[:, ::-1, :]
        .reshape(128, XT_COLS)
    )
    return np.ascontiguousarray(xt)


def _prep_inputs(x, blocks, dt_name):
    """Host-side fold + layout prep (numpy ops on the small inputs)."""
    x = np.ascontiguousarray(np.asarray(x), dtype=np.float32)
    blocks = np.ascontiguousarray(np.asarray(blocks), dtype=np.float32)
    wt_name, mv_name = _split_dt(dt_name)
    np_w, np_m = _np_dtype(wt_name), _np_dtype(mv_name)

    xc = x[:, : 16 * L] + x[:, 16 * L:]
    xs = x[:, : 16 * L] - x[:, 16 * L:]
    bc = blocks[:L] + blocks[L:]
    bs = blocks[:L] - blocks[L:]

    xt_c = _xt_layout(xc).astype(np_m)
    xt_s = _xt_layout(xs).astype(np_m)

    u = np.arange(W)
    ni = np.arange(8)
    j = u[None, :] - ni[:, None]                    # [8, W], in [-7, W-1]
    in_maps = []
    for k in range(NCORES):
        neg = k >= 4
        m0 = (k % 4) * MBLK
        jj = m0 + j                                  # in [-7, 511]
        idx = jj % L
        bh = bs if neg else bc
        bigq = bh[idx]                               # [8, W, p, q]
        if neg:
            # window index jj = (m-n) + L, so wrap sign flips at jj == L
            sgn = np.where(jj >= L, 1.0, -1.0).astype(np.float32)
            bigq = bigq * sgn[:, :, None, None]
        bigq = bigq.transpose(0, 3, 1, 2).reshape(128, W * 16)  # [(ni,q),(u,p)]
        in_maps.append({
            "xt": xt_s if neg else xt_c,
            "bigq": np.ascontiguousarray(bigq.astype(np_w)),
        })
    return in_maps


def _assemble(results):
    """Per-core [128 (mi,p), 256 (t',b)] -> slabs -> CRT unfold."""
    yc = np.empty((B, 16 * L), dtype=np.float32)
    ys = np.empty((B, 16 * L), dtype=np.float32)
    for k in range(NCORES):
        o = np.asarray(results[k]["out"]).astype(np.float32)
        slab = (
            o.reshape(128, 8, B)[:, ::-1, :].transpose(2, 1, 0).reshape(B, 1024)
        )
        dst = ys if k >= 4 else yc
        kk = k % 4
        dst[:, 1024 * kk: 1024 * (kk + 1)] = slab
    y = np.empty((B, NB * 16), dtype=np.float32)
    y[:, : 16 * L] = 0.5 * (yc + ys)
    y[:, 16 * L:] = 0.5 * (yc - ys)
    return y


def kernel(x, blocks):
    global _last_results
    from concourse.bass_utils import run_bass_kernel_spmd

    nc = _get_program(DTYPE)
    in_maps = _prep_inputs(x, blocks, DTYPE)
    res = run_bass_kernel_spmd(nc, in_maps, list(range(NCORES)))
    _last_results = res
    return _assemble(res.results)
